# revision 27
# baseline (speedup 1.0000x reference)
"""Trainium2 Bass kernel for nn_DecoderWithAttention (LSTM decoder + vocab head).

Strategy (8 NeuronCores, SPMD — identical program, per-core data differs only
in the w_fc vocab shard):
  * Host: sort batch by length (descending), gather token embeddings.
  * Phase 0: h0/c0 init projections  [64,2048]@[2048,512]
  * Phase 1: X = embs @ w_ih.T + (b_ih+b_hh) for all 50 steps as one batched
    GEMM -> DRAM scratch (streamed back per step).
  * Recurrence: 50 LSTM steps, batch-major [64, *]; gates accumulate in PSUM
    (h@w_hh.T via fp32r matmuls, X added via identity matmul); sigmoid/tanh
    on ACT; cell update on DVE; h transposed on PE into a rolling
    feature-major pair buffer.
  * Phase 2 (interleaved): vocab projection of each step-pair's h against the
    SBUF-resident w_fc shard [512, 3750]; valid-prefix rows DMA'd to the
    pre-zeroed output (rows past dec_len stay exactly 0).
  * Host: concatenate the 8 vocab shards.

Self-contained: hardcodes all shapes; only needs /opt/trn_rl_repo on the path
for the concourse (Bass/Tile) runtime.
"""

import sys

for _p in ("/opt/trn_rl_repo",):
    if _p not in sys.path:
        sys.path.insert(0, _p)

import numpy as np

import concourse.bass as bass
import concourse.bacc as bacc
import concourse.mybir as mybir
import concourse.tile as tile
from concourse.bass_utils import run_bass_kernel_spmd

F32 = mybir.dt.float32
F32R = mybir.dt.float32r
AF = mybir.ActivationFunctionType

B, L, E, H, V, ENC = 64, 51, 512, 512, 30000, 2048
T = L - 1                      # 50 decode steps
NCORES = 8
VS = V // NCORES               # 3750 vocab rows per core
# fp32r matmuls need an even moving free dim >=256 for full rate, so the
# 3750-wide vocab shard is cut into 8 even chunks.
VCHS = [470, 470, 470, 470, 470, 466, 468, 466]
VOFF = [sum(VCHS[:i]) for i in range(len(VCHS))]
NVC = len(VCHS)
KH = H // 128                  # 4 contraction chunks of the hidden dim
KE = ENC // 128                # 16 contraction chunks of the encoder dim
R = T // 2                     # 25 step-pair row tiles of 128
ROWS = T * B                   # 3200 (t-major token rows)


def _r32(ap):
    return ap.bitcast(F32R)


def _pack_schedule(bv):
    """Static packing of valid rows (t-major, prefix-valid) into 128-col
    phase-2 tiles. Returns (csum, tiles) where tiles[ti] is a dict with
    'cols' (used columns) and 'runs' = [(s, rb0, rb1, colbase), ...]."""
    csum = [0]
    for s in range(T):
        csum.append(csum[-1] + min(bv[s], 64))
    nv_total = csum[-1]
    ntiles = (nv_total + 127) // 128
    tiles = []
    for ti in range(ntiles):
        lo, hi = ti * 128, min((ti + 1) * 128, nv_total)
        runs = []
        for s in range(T):
            a, b = max(csum[s], lo), min(csum[s + 1], hi)
            if a < b:
                runs.append((s, a - csum[s], b - csum[s], a - lo))
        tiles.append({"cols": hi - lo, "runs": runs})
    return csum, tiles


def _build_program(bv):
    """Emit the full SPMD Tile program. bv[s] = #rows still alive at step s."""
    from contextlib import ExitStack

    csum, ptiles = _pack_schedule(bv)
    ntiles = len(ptiles)

    nc = bacc.Bacc("TRN2", target_bir_lowering=False, debug=False,
                   enable_asserts=False, num_devices=NCORES)

    din = {}
    for name, shape in (
        ("embsT", [128, KH, ROWS]),
        ("w_ihT", [128, KH, 4 * H]),
        ("w_hhT", [128, KH, 4 * H]),
        ("w_fcT", [128, KH, VS]),
        ("encT", [128, KE, B]),
        ("winhT", [KE, 128, H]),
        ("wincT", [KE, 128, H]),
        ("bias_g", [1, 4 * H]),
        ("binh", [1, H]),
        ("binc", [1, H]),
        ("ident64", [64, 64]),
    ):
        din[name] = nc.dram_tensor(name, shape, F32, kind="ExternalInput").ap()
    out = nc.dram_tensor("preds", [T, B, VS], F32, kind="ExternalOutput").ap()
    xdram = [nc.dram_tensor(f"x_scratch_{r}", [4, 128, 512], F32,
                            kind="Internal").ap() for r in range(R)]

    with tile.TileContext(nc) as tc, ExitStack() as es:
        pool = tc.tile_pool
        constp = es.enter_context(pool(name="const", bufs=1))
        wfcp = es.enter_context(pool(name="wfc", bufs=1))
        stagep = es.enter_context(pool(name="stage", bufs=2))
        ph1wp = es.enter_context(pool(name="ph1w", bufs=1))
        cstp = es.enter_context(pool(name="cst", bufs=1))
        hstp = es.enter_context(pool(name="hst", bufs=2))
        ps4 = es.enter_context(pool(name="ps4", bufs=4, space="PSUM"))
        ps2 = es.enter_context(pool(name="ps2", bufs=2, space="PSUM"))
        psv = es.enter_context(pool(name="psv", bufs=2, space="PSUM"))

        ident = constp.tile([64, 64], F32, tag="ident")
        nc.sync.dma_start(ident[:], din["ident64"][:])
        ones_fsrc = constp.tile([1, 128], F32, tag="onesf")
        nc.gpsimd.memset(ones_fsrc[:], 1.0)
        ones1 = constp.tile([1, 128], F32R, tag="ones1")
        nc.vector.tensor_copy(ones1[:], ones_fsrc[:])

        # ---- phase 0 first: h0 / c0 (plain fp32, scoped pool) -----------
        with pool(name="ph0", bufs=3) as ph0p:
            encT = ph0p.tile([128, KE, B], F32, tag="encT", bufs=1)
            nc.sync.dma_start(encT[:], din["encT"][:])
            bin_h = ph0p.tile([1, H], F32, tag="binh", bufs=1)
            nc.sync.dma_start(bin_h[:], din["binh"][:])
            bin_c = ph0p.tile([1, H], F32, tag="binc", bufs=1)
            nc.sync.dma_start(bin_c[:], din["binc"][:])

            ps_h = ps2.tile([128, 512], F32, tag="psA")
            ps_c = ps2.tile([128, 512], F32, tag="psA")
            for k in range(KE):
                wh = ph0p.tile([128, H], F32, tag="ph0w")
                nc.sync.dma_start(wh[:], din["winhT"][k])
                wc = ph0p.tile([128, H], F32, tag="ph0w")
                nc.sync.dma_start(wc[:], din["wincT"][k])
                nc.tensor.matmul(ps_h[:64, :], encT[:, k, :], wh[:],
                                 start=(k == 0), stop=False)
                nc.tensor.matmul(ps_c[:64, :], encT[:, k, :], wc[:],
                                 start=(k == 0), stop=False)
            ones_f = ones_fsrc
            nc.tensor.matmul(ps_h[:64, :], ones_f[:, :64], bin_h[:],
                             start=False, stop=True)
            nc.tensor.matmul(ps_c[:64, :], ones_f[:, :64], bin_c[:],
                             start=False, stop=True)
            c_sb = cstp.tile([64, H], F32, tag="c")
            nc.vector.tensor_copy(c_sb[:], ps_c[:64, :])
            h_sb = hstp.tile([64, H], F32, tag="h")
            nc.vector.tensor_copy(h_sb[:], ps_h[:64, :])

            h0T = constp.tile([128, KH, 64], F32R, tag="h0T")
            tp0 = ps2.tile([128, KH * 64], F32, tag="psA")
            for k in range(KH):
                nc.tensor.matmul(tp0[:, k * 64:(k + 1) * 64],
                                 h_sb[:, k * 128:(k + 1) * 128], ident[:],
                                 is_transpose=True,
                                 start=(k == 0), stop=(k == KH - 1))
            nc.vector.tensor_copy(h0T[:],
                                  tp0[:].rearrange("p (k b) -> p k b", k=KH))

        # ---- weights: DMA to fp32 staging, engine-copy to fp32r ---------
        w_hh_sb = constp.tile([128, KH, 4 * H], F32R, tag="whh")
        w_fc_sb = wfcp.tile([128, KH, VS], F32R, tag="wfc")
        w_ih_sb = ph1wp.tile([128, KH, 4 * H], F32R, tag="wih")
        for dst, src_, nch in ((w_hh_sb, din["w_hhT"], 8),
                               (w_ih_sb, din["w_ihT"], 8),
                               (w_fc_sb, din["w_fcT"], 15)):
            w = dst.shape[2] // nch
            for j in range(nch):
                stg = stagep.tile([128, KH, 256], F32, tag="stg")
                nc.sync.dma_start(stg[:, :, :w], src_[:, :, j * w:(j + 1) * w])
                nc.scalar.copy(dst[:, :, j * w:(j + 1) * w], stg[:, :, :w])

        stgb = stagep.tile([1, 4 * H], F32, tag="stgb", bufs=1)
        nc.sync.dma_start(stgb[:], din["bias_g"][:])
        bias_g = ph1wp.tile([1, 4 * H], F32R, tag="biasg")
        nc.vector.tensor_copy(bias_g[:], stgb[:])

        # ---- recurrence-era pools (reuse the ph0 zone) ------------------
        emtp = es.enter_context(pool(name="emt", bufs=2))
        xoutp = es.enter_context(pool(name="xout", bufs=2))
        xinp = es.enter_context(pool(name="xin", bufs=2))
        gatep = es.enter_context(pool(name="gate", bufs=1))
        hsTp = es.enter_context(pool(name="hsT", bufs=3))
        packp = es.enter_context(pool(name="pack", bufs=4))
        p2op = es.enter_context(pool(name="p2o", bufs=2))

        def phase1_tile(r):
            """X[128 rows, 2048] for step pair r -> DRAM scratch."""
            em_f = emtp.tile([128, KH, 128], F32, tag="emf", bufs=1)
            nc.sync.dma_start(em_f[:], din["embsT"][:, :, 128 * r:128 * (r + 1)])
            em = emtp.tile([128, KH, 128], F32R, tag="em")
            nc.vector.tensor_copy(em[:], em_f[:])
            for j in range(2):
                xo = xoutp.tile([128, 2, 512], F32, tag="xo")
                for h2 in range(2):
                    n = 2 * j + h2
                    ps = ps2.tile([128, 512], F32, tag="psA")
                    for k in range(KH):
                        nc.tensor.matmul(ps[:], em[:, k, :],
                                         w_ih_sb[:, k, 512 * n:512 * (n + 1)],
                                         start=(k == 0), stop=False)
                    nc.tensor.matmul(ps[:], ones1[:],
                                     bias_g[:, 512 * n:512 * (n + 1)],
                                     start=False, stop=True)
                    nc.vector.tensor_copy(xo[:, h2, :], ps[:])
                nc.sync.dma_start(
                    xdram[r][2 * j:2 * j + 2].rearrange("n p c -> p n c"),
                    xo[:])

        pack_tiles = {}
        state = {"h": h_sb, "c": c_sb, "stat": h0T}

        def step(s):
            """One LSTM step (batch-major); h -> stationary + pack tiles."""
            stat = state["stat"]

            xt = xinp.tile([64, KH, 512], F32, tag="xin")
            h64 = (s % 2) * 64
            nc.sync.dma_start(
                xt[:], xdram[s // 2][:, h64:h64 + 64, :].rearrange(
                    "n b c -> b n c"))

            # gate order: f, i, g, o  (w_hh columns: i|f|g|o)
            gps = {}
            for g in (1, 0, 2, 3):
                ps = ps4.tile([64, 512], F32, tag="gps")
                for k in range(KH):
                    nc.tensor.matmul(
                        ps[:], stat[:, k, :],
                        w_hh_sb[:, k, 512 * g:512 * (g + 1)],
                        start=(k == 0), stop=(k == KH - 1))
                # X (with biases folded in) added on DVE straight into psum
                nc.vector.tensor_add(ps[:], ps[:], xt[:, g, :])
                gps[g] = ps

            c_sb = state["c"]
            f_sb = gatep.tile([64, H], F32, tag="f")
            nc.scalar.activation(f_sb[:], gps[1][:], AF.Sigmoid)
            i_sb = gatep.tile([64, H], F32, tag="i")
            nc.scalar.activation(i_sb[:], gps[0][:], AF.Sigmoid)
            g_sb = gatep.tile([64, H], F32, tag="g")
            nc.scalar.activation(g_sb[:], gps[2][:], AF.Tanh)
            o_sb = gatep.tile([64, H], F32, tag="o")
            nc.scalar.activation(o_sb[:], gps[3][:], AF.Sigmoid)

            nc.vector.tensor_mul(f_sb[:], f_sb[:], c_sb[:])   # f*c
            nc.vector.tensor_mul(i_sb[:], i_sb[:], g_sb[:])   # i*g
            c_new = cstp.tile([64, H], F32, tag="c")
            nc.vector.tensor_add(c_new[:], f_sb[:], i_sb[:])
            state["c"] = c_new
            nc.scalar.activation(g_sb[:], c_new[:], AF.Tanh)  # tanh(c)
            h_new = hstp.tile([64, H], F32, tag="h")
            nc.vector.tensor_mul(h_new[:], o_sb[:], g_sb[:])
            state["h"] = h_new

            tp = ps2.tile([128, KH * 64], F32, tag="psA")
            for k in range(KH):
                nc.tensor.matmul(tp[:, k * 64:(k + 1) * 64],
                                 h_new[:, k * 128:(k + 1) * 128], ident[:],
                                 is_transpose=True,
                                 start=(k == 0), stop=(k == KH - 1))
            src = tp[:].rearrange("p (k b) -> p k b", k=KH)
            hsT = hsTp.tile([128, KH, 64], F32R, tag="hsT")
            nc.vector.tensor_copy(hsT[:], src)
            state["stat"] = hsT

            # scatter valid rows into packed phase-2 tiles
            nv = min(bv[s], 64)
            pos = csum[s]
            done = 0
            while done < nv:
                ti = (pos + done) // 128
                col = (pos + done) % 128
                take = min(nv - done, 128 - col)
                if ti not in pack_tiles:
                    pack_tiles[ti] = packp.tile(
                        [128, KH, 128], F32R, tag="pack", name=f"pack{ti}")
                nc.vector.tensor_copy(
                    pack_tiles[ti][:, :, col:col + take],
                    src[:, :, done:done + take])
                done += take

        def phase2_chunk(ti, n):
            """One vocab chunk of one packed 128-row tile."""
            info = ptiles[ti]
            cols = info["cols"]
            ptile = pack_tiles[ti]
            w, off = VCHS[n], VOFF[n]
            ps = psv.tile([128, 470], F32, tag="pv")
            for k in range(KH):
                nc.tensor.matmul(
                    ps[:cols, :w], ptile[:, k, :cols],
                    w_fc_sb[:, k, off:off + w],
                    start=(k == 0), stop=(k == KH - 1))
            ob = p2op.tile([128, 470], F32, tag="p2o")
            if n % 2 == 0:
                nc.vector.tensor_copy(ob[:cols, :w], ps[:cols, :w])
            else:
                nc.scalar.copy(ob[:cols, :w], ps[:cols, :w])
            for (s, rb0, rb1, colbase) in info["runs"]:
                nc.sync.dma_start(
                    out[s, rb0:rb1, off:off + w],
                    ob[colbase:colbase + (rb1 - rb0), :w])
            if n == NVC - 1:
                pack_tiles.pop(ti)

        # ---- emission: pipeline phase 1 / recurrence / phase 2 ----------
        # pack tile ti completes during the step s with csum[s+1] >= (ti+1)*128
        done_after = {}
        for ti in range(ntiles):
            for s in range(T):
                if csum[s + 1] >= min((ti + 1) * 128, csum[-1]):
                    done_after[s] = done_after.get(s, []) + [ti]
                    break

        phase1_tile(0)
        phase1_tile(1)
        from collections import deque
        pending = deque()
        for s in range(T):
            # pace phase 1 at consumption rate (one pair-tile per 2 steps,
            # 2 tiles ahead) so its PE work also fills the late-step gaps
            if s % 2 == 0 and s // 2 + 2 < R:
                phase1_tile(s // 2 + 2)
            step(s)
            for ti in done_after.get(s, []):
                pending.extend((ti, n) for n in range(NVC))
            # meter phase-2 chunks so PE filler work spans the whole
            # recurrence instead of being consumed up front
            quota = 2 if s < 25 else 3
            for _ in range(min(quota, len(pending))):
                phase2_chunk(*pending.popleft())
        while pending:
            phase2_chunk(*pending.popleft())

    nc.compile()
    return nc


_CACHE = {}
LAST_RESULTS = None


def kernel(**inputs):
    x = {k: np.asarray(v) for k, v in inputs.items()}
    enc = np.ascontiguousarray(x["encoder_out"], dtype=np.float32)
    caps = x["encoded_captions"]
    lengths = x["caption_lengths"][:, 0]
    emb_w = np.ascontiguousarray(x["embedding_weight"], dtype=np.float32)
    w_ih = x["w_ih"].astype(np.float32, copy=False)
    b_ih = x["b_ih"].astype(np.float32, copy=False)
    w_hh = x["w_hh"].astype(np.float32, copy=False)
    b_hh = x["b_hh"].astype(np.float32, copy=False)
    w_init_h = x["w_init_h"].astype(np.float32, copy=False)
    b_init_h = x["b_init_h"].astype(np.float32, copy=False)
    w_init_c = x["w_init_c"].astype(np.float32, copy=False)
    b_init_c = x["b_init_c"].astype(np.float32, copy=False)
    w_fc = x["w_fc"].astype(np.float32, copy=False)
    b_fc = x["b_fc"].astype(np.float32, copy=False)

    sort_ind = np.argsort(-lengths.astype(np.int64), kind="stable")
    enc_s = enc[sort_ind]
    caps_s = caps[sort_ind]
    dec_len = (lengths[sort_ind].astype(np.int64) - 1)
    bv = [int((dec_len > s).sum()) for s in range(T)]

    toks = np.asarray(caps_s[:, :T], dtype=np.int64)
    embs = emb_w[toks]                                   # [B, T, E]
    em = np.ascontiguousarray(
        embs.transpose(1, 0, 2).reshape(ROWS, E))        # row t*64+b

    def kchunk(mat_t):  # [D, N] -> [128, D//128, N]
        d = mat_t.shape[0]
        return np.ascontiguousarray(
            mat_t.reshape(d // 128, 128, -1).transpose(1, 0, 2))

    feed = {
        "embsT": kchunk(em.T),
        "w_ihT": kchunk(w_ih.T),
        "w_hhT": kchunk(w_hh.T),
        "encT": kchunk(enc_s.T),
        "winhT": np.ascontiguousarray(w_init_h.T.reshape(KE, 128, H)),
        "wincT": np.ascontiguousarray(w_init_c.T.reshape(KE, 128, H)),
        "bias_g": (b_ih + b_hh).reshape(1, -1),
        "binh": b_init_h.reshape(1, -1),
        "binc": b_init_c.reshape(1, -1),
        "ident64": np.eye(64, dtype=np.float32),
    }
    feed = {k: np.ascontiguousarray(v, dtype=np.float32) for k, v in feed.items()}

    key = tuple(bv)
    if key not in _CACHE:
        _CACHE[key] = _build_program(bv)
    nc = _CACHE[key]

    in_maps = []
    for c in range(NCORES):
        m = dict(feed)
        m["w_fcT"] = kchunk(np.ascontiguousarray(w_fc[c * VS:(c + 1) * VS].T))
        in_maps.append(m)

    res = run_bass_kernel_spmd(nc, in_maps, core_ids=list(range(NCORES)))
    global LAST_RESULTS
    LAST_RESULTS = res
    shards = [res.results[c]["preds"].transpose(1, 0, 2)
              for c in range(NCORES)]
    preds = np.concatenate(shards, axis=2)

    if b_fc.any():
        mask = np.arange(T)[None, :] < dec_len[:, None]
        preds = preds + np.where(mask[:, :, None], b_fc[None, None, :], 0.0)

    int_dt = caps.dtype if caps.dtype in (np.int32, np.int64) else np.int64
    return (preds,
            caps_s.astype(int_dt, copy=False),
            dec_len.astype(x["caption_lengths"].dtype, copy=False),
            sort_ind.astype(np.int32))


# revision 28
# speedup vs baseline: 3.1296x; 3.1296x over previous
"""Trainium2 Bass kernel for nn_DecoderWithAttention (LSTM decoder + vocab head).

Strategy (8 NeuronCores, SPMD — identical program, per-core data differs only
in the w_fc vocab shard):
  * Host: sort batch by length (descending), gather token embeddings.
  * Phase 0: h0/c0 init projections  [64,2048]@[2048,512]
  * Phase 1: X = embs @ w_ih.T + (b_ih+b_hh) for all 50 steps as one batched
    GEMM -> DRAM scratch (streamed back per step).
  * Recurrence: 50 LSTM steps, batch-major [64, *]; gates accumulate in PSUM
    (h@w_hh.T via fp32r matmuls, X added via identity matmul); sigmoid/tanh
    on ACT; cell update on DVE; h transposed on PE into a rolling
    feature-major pair buffer.
  * Phase 2 (interleaved): vocab projection of each step-pair's h against the
    SBUF-resident w_fc shard [512, 3750]; valid-prefix rows DMA'd to the
    pre-zeroed output (rows past dec_len stay exactly 0).
  * Host: concatenate the 8 vocab shards.

Self-contained: hardcodes all shapes; only needs /opt/trn_rl_repo on the path
for the concourse (Bass/Tile) runtime.
"""

import sys

for _p in ("/opt/trn_rl_repo",):
    if _p not in sys.path:
        sys.path.insert(0, _p)

import numpy as np

import concourse.bass as bass
import concourse.bacc as bacc
import concourse.mybir as mybir
import concourse.tile as tile
from concourse.bass_utils import run_bass_kernel_spmd

F32 = mybir.dt.float32
F32R = mybir.dt.float32r
AF = mybir.ActivationFunctionType

B, L, E, H, V, ENC = 64, 51, 512, 512, 30000, 2048
T = L - 1                      # 50 decode steps
NCORES = 8
VS = V // NCORES               # 3750 vocab rows per core
# fp32r matmuls need an even moving free dim >=256 for full rate, so the
# 3750-wide vocab shard is cut into 8 even chunks.
VCHS = [470, 470, 470, 470, 470, 466, 468, 466]
VOFF = [sum(VCHS[:i]) for i in range(len(VCHS))]
NVC = len(VCHS)
KH = H // 128                  # 4 contraction chunks of the hidden dim
KE = ENC // 128                # 16 contraction chunks of the encoder dim
R = T // 2                     # 25 step-pair row tiles of 128
ROWS = T * B                   # 3200 (t-major token rows)


def _r32(ap):
    return ap.bitcast(F32R)


def _pack_schedule(bv):
    """Static packing of valid rows (t-major, prefix-valid) into 128-col
    phase-2 tiles. Returns (csum, tiles) where tiles[ti] is a dict with
    'cols' (used columns) and 'runs' = [(s, rb0, rb1, colbase), ...]."""
    csum = [0]
    for s in range(T):
        csum.append(csum[-1] + min(bv[s], 64))
    nv_total = csum[-1]
    ntiles = (nv_total + 127) // 128
    tiles = []
    for ti in range(ntiles):
        lo, hi = ti * 128, min((ti + 1) * 128, nv_total)
        runs = []
        for s in range(T):
            a, b = max(csum[s], lo), min(csum[s + 1], hi)
            if a < b:
                runs.append((s, a - csum[s], b - csum[s], a - lo))
        tiles.append({"cols": hi - lo, "runs": runs})
    return csum, tiles


def _build_program(bv):
    """Emit the full SPMD Tile program. bv[s] = #rows still alive at step s."""
    from contextlib import ExitStack

    csum, ptiles = _pack_schedule(bv)
    ntiles = len(ptiles)

    nc = bacc.Bacc("TRN2", target_bir_lowering=False, debug=False,
                   enable_asserts=False, num_devices=NCORES)

    din = {}
    for name, shape in (
        ("embsT", [128, KH, ROWS]),
        ("w_ihT", [128, KH, 4 * H]),
        ("w_hhT", [128, KH, 4 * H]),
        ("w_fcT", [128, KH, VS]),
        ("encT", [128, KE, B]),
        ("winhT", [KE, 128, H]),
        ("wincT", [KE, 128, H]),
        ("bias_g", [1, 4 * H]),
        ("binh", [1, H]),
        ("binc", [1, H]),
        ("ident64", [64, 64]),
    ):
        din[name] = nc.dram_tensor(name, shape, F32, kind="ExternalInput").ap()
    out = nc.dram_tensor("preds", [T, B, VS], F32, kind="ExternalOutput").ap()
    xdram = [nc.dram_tensor(f"x_scratch_{r}", [4, 128, 512], F32,
                            kind="Internal").ap() for r in range(R)]

    with tile.TileContext(nc) as tc, ExitStack() as es:
        pool = tc.tile_pool
        constp = es.enter_context(pool(name="const", bufs=1))
        wfcp = es.enter_context(pool(name="wfc", bufs=1))
        stagep = es.enter_context(pool(name="stage", bufs=2))
        ph1wp = es.enter_context(pool(name="ph1w", bufs=1))
        cstp = es.enter_context(pool(name="cst", bufs=1))
        hstp = es.enter_context(pool(name="hst", bufs=2))
        ps4 = es.enter_context(pool(name="ps4", bufs=4, space="PSUM"))
        ps2 = es.enter_context(pool(name="ps2", bufs=2, space="PSUM"))
        psv = es.enter_context(pool(name="psv", bufs=2, space="PSUM"))

        ident = constp.tile([64, 64], F32, tag="ident")
        nc.sync.dma_start(ident[:], din["ident64"][:])
        ones_fsrc = constp.tile([1, 128], F32, tag="onesf")
        nc.gpsimd.memset(ones_fsrc[:], 1.0)
        ones1 = constp.tile([1, 128], F32R, tag="ones1")
        nc.vector.tensor_copy(ones1[:], ones_fsrc[:])

        # ---- phase 0 first: h0 / c0 (plain fp32, scoped pool) -----------
        with pool(name="ph0", bufs=3) as ph0p:
            encT = ph0p.tile([128, KE, B], F32, tag="encT", bufs=1)
            nc.sync.dma_start(encT[:], din["encT"][:])
            bin_h = ph0p.tile([1, H], F32, tag="binh", bufs=1)
            nc.sync.dma_start(bin_h[:], din["binh"][:])
            bin_c = ph0p.tile([1, H], F32, tag="binc", bufs=1)
            nc.sync.dma_start(bin_c[:], din["binc"][:])

            ps_h = ps2.tile([128, 512], F32, tag="psA")
            ps_c = ps2.tile([128, 512], F32, tag="psA")
            for k in range(KE):
                wh = ph0p.tile([128, H], F32, tag="ph0w")
                nc.sync.dma_start(wh[:], din["winhT"][k])
                wc = ph0p.tile([128, H], F32, tag="ph0w")
                nc.sync.dma_start(wc[:], din["wincT"][k])
                nc.tensor.matmul(ps_h[:64, :], encT[:, k, :], wh[:],
                                 start=(k == 0), stop=False)
                nc.tensor.matmul(ps_c[:64, :], encT[:, k, :], wc[:],
                                 start=(k == 0), stop=False)
            ones_f = ones_fsrc
            nc.tensor.matmul(ps_h[:64, :], ones_f[:, :64], bin_h[:],
                             start=False, stop=True)
            nc.tensor.matmul(ps_c[:64, :], ones_f[:, :64], bin_c[:],
                             start=False, stop=True)
            c_sb = cstp.tile([64, H], F32, tag="c")
            nc.vector.tensor_copy(c_sb[:], ps_c[:64, :])
            h_sb = hstp.tile([64, H], F32, tag="h")
            nc.vector.tensor_copy(h_sb[:], ps_h[:64, :])

            h0T = constp.tile([128, KH, 64], F32R, tag="h0T")
            tp0 = ps2.tile([128, KH * 64], F32, tag="psA")
            for k in range(KH):
                nc.tensor.matmul(tp0[:, k * 64:(k + 1) * 64],
                                 h_sb[:, k * 128:(k + 1) * 128], ident[:],
                                 is_transpose=True,
                                 start=(k == 0), stop=(k == KH - 1))
            nc.vector.tensor_copy(h0T[:],
                                  tp0[:].rearrange("p (k b) -> p k b", k=KH))

        # ---- weights: DMA to fp32 staging, engine-copy to fp32r ---------
        w_hh_sb = constp.tile([128, KH, 4 * H], F32R, tag="whh")
        w_fc_sb = wfcp.tile([128, KH, VS], F32R, tag="wfc")
        w_ih_sb = ph1wp.tile([128, KH, 4 * H], F32R, tag="wih")
        for dst, src_, nch in ((w_hh_sb, din["w_hhT"], 8),
                               (w_ih_sb, din["w_ihT"], 8),
                               (w_fc_sb, din["w_fcT"], 15)):
            w = dst.shape[2] // nch
            for j in range(nch):
                stg = stagep.tile([128, KH, 256], F32, tag="stg")
                nc.sync.dma_start(stg[:, :, :w], src_[:, :, j * w:(j + 1) * w])
                nc.scalar.copy(dst[:, :, j * w:(j + 1) * w], stg[:, :, :w])

        stgb = stagep.tile([1, 4 * H], F32, tag="stgb", bufs=1)
        nc.sync.dma_start(stgb[:], din["bias_g"][:])
        bias_g = ph1wp.tile([1, 4 * H], F32R, tag="biasg")
        nc.vector.tensor_copy(bias_g[:], stgb[:])

        # ---- recurrence-era pools (reuse the ph0 zone) ------------------
        emtp = es.enter_context(pool(name="emt", bufs=2))
        xoutp = es.enter_context(pool(name="xout", bufs=2))
        xinp = es.enter_context(pool(name="xin", bufs=2))
        gatep = es.enter_context(pool(name="gate", bufs=1))
        hTp = es.enter_context(pool(name="hT", bufs=4))
        p2op = es.enter_context(pool(name="p2o", bufs=2))

        def phase1_tile(r):
            """X[128 rows, 2048] for step pair r -> DRAM scratch."""
            em_f = emtp.tile([128, KH, 128], F32, tag="emf", bufs=1)
            nc.sync.dma_start(em_f[:], din["embsT"][:, :, 128 * r:128 * (r + 1)])
            em = emtp.tile([128, KH, 128], F32R, tag="em")
            nc.vector.tensor_copy(em[:], em_f[:])
            for j in range(2):
                xo = xoutp.tile([128, 2, 512], F32, tag="xo")
                for h2 in range(2):
                    n = 2 * j + h2
                    ps = ps2.tile([128, 512], F32, tag="psA")
                    for k in range(KH):
                        nc.tensor.matmul(ps[:], em[:, k, :],
                                         w_ih_sb[:, k, 512 * n:512 * (n + 1)],
                                         start=(k == 0), stop=False)
                    nc.tensor.matmul(ps[:], ones1[:],
                                     bias_g[:, 512 * n:512 * (n + 1)],
                                     start=False, stop=True)
                    nc.vector.tensor_copy(xo[:, h2, :], ps[:])
                nc.sync.dma_start(
                    xdram[r][2 * j:2 * j + 2].rearrange("n p c -> p n c"),
                    xo[:])

        hT_tiles = {}
        state = {"h": h_sb, "c": c_sb}

        def step(s):
            """One LSTM step (batch-major); h -> feature-major pair tile."""
            if s % 2 == 0:
                hT_tiles[s // 2] = hTp.tile([128, KH, 128], F32R,
                                            tag="hTpair",
                                            name=f"hTpair{s // 2}")
            pair = hT_tiles[s // 2]
            half = (s % 2) * 64
            stat_t = h0T if s == 0 else hT_tiles[(s - 1) // 2]
            soff = 0 if s == 0 else ((s - 1) % 2) * 64
            stat = stat_t[:, :, soff:soff + 64] if stat_t is not h0T else h0T[:]

            xt = xinp.tile([64, KH, 512], F32, tag="xin")
            h64 = (s % 2) * 64
            nc.sync.dma_start(
                xt[:], xdram[s // 2][:, h64:h64 + 64, :].rearrange(
                    "n b c -> b n c"))

            # gate order: f, i, g, o  (w_hh columns: i|f|g|o)
            gps = {}
            for g in (1, 0, 2, 3):
                ps = ps4.tile([64, 512], F32, tag="gps")
                for k in range(KH):
                    nc.tensor.matmul(
                        ps[:], stat_t[:, k, soff:soff + 64],
                        w_hh_sb[:, k, 512 * g:512 * (g + 1)],
                        start=(k == 0), stop=(k == KH - 1))
                # X (with biases folded in) added on DVE straight into psum
                nc.vector.tensor_add(ps[:], ps[:], xt[:, g, :])
                gps[g] = ps

            c_sb = state["c"]
            f_sb = gatep.tile([64, H], F32, tag="f")
            nc.scalar.activation(f_sb[:], gps[1][:], AF.Sigmoid)
            i_sb = gatep.tile([64, H], F32, tag="i")
            nc.scalar.activation(i_sb[:], gps[0][:], AF.Sigmoid)
            g_sb = gatep.tile([64, H], F32, tag="g")
            nc.scalar.activation(g_sb[:], gps[2][:], AF.Tanh)
            o_sb = gatep.tile([64, H], F32, tag="o")
            nc.scalar.activation(o_sb[:], gps[3][:], AF.Sigmoid)

            nc.vector.tensor_mul(f_sb[:], f_sb[:], c_sb[:])   # f*c
            nc.vector.tensor_mul(i_sb[:], i_sb[:], g_sb[:])   # i*g
            c_new = cstp.tile([64, H], F32, tag="c")
            nc.vector.tensor_add(c_new[:], f_sb[:], i_sb[:])
            state["c"] = c_new
            nc.scalar.activation(g_sb[:], c_new[:], AF.Tanh)  # tanh(c)
            h_new = hstp.tile([64, H], F32, tag="h")
            nc.vector.tensor_mul(h_new[:], o_sb[:], g_sb[:])
            state["h"] = h_new

            tp = ps2.tile([128, KH * 64], F32, tag="psA")
            for k in range(KH):
                nc.tensor.matmul(tp[:, k * 64:(k + 1) * 64],
                                 h_new[:, k * 128:(k + 1) * 128], ident[:],
                                 is_transpose=True,
                                 start=(k == 0), stop=(k == KH - 1))
            src = tp[:].rearrange("p (k b) -> p k b", k=KH)
            nv = min(bv[s], 64)
            if nv >= 64:
                nc.vector.tensor_copy(pair[:, :, half:half + 64], src)
            else:
                if nv > 0:
                    nc.vector.tensor_copy(pair[:, :, half:half + nv],
                                          src[:, :, :nv])
                nc.gpsimd.memset(
                    pair[:, :, half + nv:half + 64].bitcast(F32), 0.0)

        def phase2_tile(r):
            """preds for steps 2r, 2r+1 against the vocab shard."""
            pair = hT_tiles.pop(r)
            for n in range(NVC):
                w, off = VCHS[n], VOFF[n]
                ps = psv.tile([128, 470], F32, tag="pv")
                for k in range(KH):
                    nc.tensor.matmul(
                        ps[:, :w], pair[:, k, :],
                        w_fc_sb[:, k, off:off + w],
                        start=(k == 0), stop=(k == KH - 1))
                ob = p2op.tile([128, 470], F32, tag="p2o")
                if n % 2 == 0:
                    nc.vector.tensor_copy(ob[:, :w], ps[:, :w])
                else:
                    nc.scalar.copy(ob[:, :w], ps[:, :w])
                for half in range(2):
                    s = 2 * r + half
                    nv = min(bv[s], 64)
                    if nv > 0:
                        nc.sync.dma_start(
                            out[s, 0:nv, off:off + w],
                            ob[64 * half:64 * half + nv, :w])

        # ---- emission: pipeline phase 1 / recurrence / phase 2 ----------
        phase1_tile(0)
        phase1_tile(1)
        for s in range(T):
            if s + 2 < R:
                phase1_tile(s + 2)
            step(s)
            if s >= 2 and s % 2 == 0:
                phase2_tile((s - 2) // 2)
        phase2_tile(R - 1)

    nc.compile()
    return nc


_CACHE = {}
LAST_RESULTS = None


def kernel(**inputs):
    x = {k: np.asarray(v) for k, v in inputs.items()}
    enc = np.ascontiguousarray(x["encoder_out"], dtype=np.float32)
    caps = x["encoded_captions"]
    lengths = x["caption_lengths"][:, 0]
    emb_w = np.ascontiguousarray(x["embedding_weight"], dtype=np.float32)
    w_ih = x["w_ih"].astype(np.float32, copy=False)
    b_ih = x["b_ih"].astype(np.float32, copy=False)
    w_hh = x["w_hh"].astype(np.float32, copy=False)
    b_hh = x["b_hh"].astype(np.float32, copy=False)
    w_init_h = x["w_init_h"].astype(np.float32, copy=False)
    b_init_h = x["b_init_h"].astype(np.float32, copy=False)
    w_init_c = x["w_init_c"].astype(np.float32, copy=False)
    b_init_c = x["b_init_c"].astype(np.float32, copy=False)
    w_fc = x["w_fc"].astype(np.float32, copy=False)
    b_fc = x["b_fc"].astype(np.float32, copy=False)

    sort_ind = np.argsort(-lengths.astype(np.int64), kind="stable")
    enc_s = enc[sort_ind]
    caps_s = caps[sort_ind]
    dec_len = (lengths[sort_ind].astype(np.int64) - 1)
    bv = [int((dec_len > s).sum()) for s in range(T)]

    toks = np.asarray(caps_s[:, :T], dtype=np.int64)
    embs = emb_w[toks]                                   # [B, T, E]
    em = np.ascontiguousarray(
        embs.transpose(1, 0, 2).reshape(ROWS, E))        # row t*64+b

    def kchunk(mat_t):  # [D, N] -> [128, D//128, N]
        d = mat_t.shape[0]
        return np.ascontiguousarray(
            mat_t.reshape(d // 128, 128, -1).transpose(1, 0, 2))

    feed = {
        "embsT": kchunk(em.T),
        "w_ihT": kchunk(w_ih.T),
        "w_hhT": kchunk(w_hh.T),
        "encT": kchunk(enc_s.T),
        "winhT": np.ascontiguousarray(w_init_h.T.reshape(KE, 128, H)),
        "wincT": np.ascontiguousarray(w_init_c.T.reshape(KE, 128, H)),
        "bias_g": (b_ih + b_hh).reshape(1, -1),
        "binh": b_init_h.reshape(1, -1),
        "binc": b_init_c.reshape(1, -1),
        "ident64": np.eye(64, dtype=np.float32),
    }
    feed = {k: np.ascontiguousarray(v, dtype=np.float32) for k, v in feed.items()}

    key = tuple(bv)
    if key not in _CACHE:
        _CACHE[key] = _build_program(bv)
    nc = _CACHE[key]

    in_maps = []
    for c in range(NCORES):
        m = dict(feed)
        m["w_fcT"] = kchunk(np.ascontiguousarray(w_fc[c * VS:(c + 1) * VS].T))
        in_maps.append(m)

    res = run_bass_kernel_spmd(nc, in_maps, core_ids=list(range(NCORES)))
    global LAST_RESULTS
    LAST_RESULTS = res
    shards = [res.results[c]["preds"].transpose(1, 0, 2)
              for c in range(NCORES)]
    preds = np.concatenate(shards, axis=2)

    if b_fc.any():
        mask = np.arange(T)[None, :] < dec_len[:, None]
        preds = preds + np.where(mask[:, :, None], b_fc[None, None, :], 0.0)

    int_dt = caps.dtype if caps.dtype in (np.int32, np.int64) else np.int64
    return (preds,
            caps_s.astype(int_dt, copy=False),
            dec_len.astype(x["caption_lengths"].dtype, copy=False),
            sort_ind.astype(np.int32))


# revision 30
# speedup vs baseline: 9.4241x; 3.0113x over previous
"""Trainium2 Bass kernel for nn_DecoderWithAttention (LSTM decoder + vocab head).

Strategy (8 NeuronCores, SPMD — identical program, per-core data differs only
in the w_fc vocab shard):
  * Host: sort batch by length (descending), gather token embeddings.
  * Phase 0: h0/c0 init projections  [64,2048]@[2048,512]
  * Phase 1: X = embs @ w_ih.T + (b_ih+b_hh) for all 50 steps as one batched
    GEMM -> DRAM scratch (streamed back per step).
  * Recurrence: 50 LSTM steps, batch-major [64, *]; gates accumulate in PSUM
    (h@w_hh.T via fp32r matmuls); X added on DVE into the psum; sigmoid/tanh
    on ACT; cell update on DVE; h transposed on PE into a rolling
    feature-major pair buffer (dead rows zeroed so their logits are 0).
  * Phase 2 (interleaved per step pair): vocab projection of each pair's h
    against the SBUF-resident w_fc shard [512, 3750] in even-width fp32r
    chunks; valid-prefix rows DMA'd to the pre-zeroed output (rows past
    dec_len stay exactly 0).
  * Host: concatenate the 8 vocab shards.

Self-contained: hardcodes all shapes; only needs /opt/trn_rl_repo on the path
for the concourse (Bass/Tile) runtime.
"""

import sys

for _p in ("/opt/trn_rl_repo",):
    if _p not in sys.path:
        sys.path.insert(0, _p)

import numpy as np

import concourse.bass as bass
import concourse.bacc as bacc
import concourse.mybir as mybir
import concourse.tile as tile
from concourse.bass_utils import run_bass_kernel_spmd

F32 = mybir.dt.float32
F32R = mybir.dt.float32r
AF = mybir.ActivationFunctionType

B, L, E, H, V, ENC = 64, 51, 512, 512, 30000, 2048
T = L - 1                      # 50 decode steps
NCORES = 8
VS = V // NCORES               # 3750 vocab rows per core
# fp32r matmuls need an even moving free dim >=256 for full rate, so the
# 3750-wide vocab shard is cut into 8 even chunks.
VCHS = [470, 470, 470, 470, 470, 466, 468, 466]
VOFF = [sum(VCHS[:i]) for i in range(len(VCHS))]
NVC = len(VCHS)
KH = H // 128                  # 4 contraction chunks of the hidden dim
KE = ENC // 128                # 16 contraction chunks of the encoder dim
R = T // 2                     # 25 step-pair row tiles of 128
ROWS = T * B                   # 3200 (t-major token rows)


def _r32(ap):
    return ap.bitcast(F32R)


def _pack_schedule(bv):
    """Static packing of valid rows (t-major, prefix-valid) into 128-col
    phase-2 tiles. Returns (csum, tiles) where tiles[ti] is a dict with
    'cols' (used columns) and 'runs' = [(s, rb0, rb1, colbase), ...]."""
    csum = [0]
    for s in range(T):
        csum.append(csum[-1] + min(bv[s], 64))
    nv_total = csum[-1]
    ntiles = (nv_total + 127) // 128
    tiles = []
    for ti in range(ntiles):
        lo, hi = ti * 128, min((ti + 1) * 128, nv_total)
        runs = []
        for s in range(T):
            a, b = max(csum[s], lo), min(csum[s + 1], hi)
            if a < b:
                runs.append((s, a - csum[s], b - csum[s], a - lo))
        tiles.append({"cols": hi - lo, "runs": runs})
    return csum, tiles


def _build_program(bv):
    """Emit the full SPMD Tile program. bv[s] = #rows still alive at step s."""
    from contextlib import ExitStack

    csum, ptiles = _pack_schedule(bv)
    ntiles = len(ptiles)

    nc = bacc.Bacc("TRN2", target_bir_lowering=False, debug=False,
                   enable_asserts=False, num_devices=NCORES)

    din = {}
    for name, shape in (
        ("embsT", [128, KH, ROWS]),
        ("w_ihT", [128, KH, 4 * H]),
        ("w_hhT", [128, KH, 4 * H]),
        ("w_fcT", [128, KH, VS]),
        ("encT", [128, KE, B]),
        ("winhT", [KE, 128, H]),
        ("wincT", [KE, 128, H]),
        ("bias_g", [1, 4 * H]),
        ("binh", [1, H]),
        ("binc", [1, H]),
        ("ident64", [64, 64]),
    ):
        din[name] = nc.dram_tensor(name, shape, F32, kind="ExternalInput").ap()
    out = nc.dram_tensor("preds", [T, B, VS], F32, kind="ExternalOutput").ap()
    xdram = [nc.dram_tensor(f"x_scratch_{r}", [4, 128, 512], F32,
                            kind="Internal").ap() for r in range(R)]

    with tile.TileContext(nc) as tc, ExitStack() as es:
        pool = tc.tile_pool
        constp = es.enter_context(pool(name="const", bufs=1))
        wfcp = es.enter_context(pool(name="wfc", bufs=1))
        stagep = es.enter_context(pool(name="stage", bufs=2))
        ph1wp = es.enter_context(pool(name="ph1w", bufs=1))
        cstp = es.enter_context(pool(name="cst", bufs=1))
        hstp = es.enter_context(pool(name="hst", bufs=2))
        ps4 = es.enter_context(pool(name="ps4", bufs=4, space="PSUM"))
        ps2 = es.enter_context(pool(name="ps2", bufs=2, space="PSUM"))
        psv = es.enter_context(pool(name="psv", bufs=2, space="PSUM"))

        ident = constp.tile([64, 64], F32, tag="ident")
        nc.sync.dma_start(ident[:], din["ident64"][:])
        ones_fsrc = constp.tile([1, 128], F32, tag="onesf")
        nc.gpsimd.memset(ones_fsrc[:], 1.0)
        ones1 = constp.tile([1, 128], F32R, tag="ones1")
        nc.vector.tensor_copy(ones1[:], ones_fsrc[:])

        # ---- phase 0 first: h0 / c0 (fp32r, scoped pool) ----------------
        with pool(name="ph0", bufs=3) as ph0p:
            encT_f = ph0p.tile([128, KE, B], F32, tag="encTf", bufs=1)
            nc.sync.dma_start(encT_f[:], din["encT"][:])
            encT = ph0p.tile([128, KE, B], F32R, tag="encT", bufs=1)
            nc.vector.tensor_copy(encT[:], encT_f[:])
            bin_h = ph0p.tile([1, H], F32, tag="binh", bufs=1)
            nc.sync.dma_start(bin_h[:], din["binh"][:])
            bin_hr = ph0p.tile([1, H], F32R, tag="binhr", bufs=1)
            nc.vector.tensor_copy(bin_hr[:], bin_h[:])
            bin_c = ph0p.tile([1, H], F32, tag="binc", bufs=1)
            nc.sync.dma_start(bin_c[:], din["binc"][:])
            bin_cr = ph0p.tile([1, H], F32R, tag="bincr", bufs=1)
            nc.vector.tensor_copy(bin_cr[:], bin_c[:])

            ps_h = ps2.tile([128, 512], F32, tag="psA")
            ps_c = ps2.tile([128, 512], F32, tag="psA")
            for k in range(KE):
                wh_f = ph0p.tile([128, H], F32, tag="ph0wf")
                nc.sync.dma_start(wh_f[:], din["winhT"][k])
                wh = ph0p.tile([128, H], F32R, tag="ph0w")
                nc.scalar.copy(wh[:], wh_f[:])
                wc_f = ph0p.tile([128, H], F32, tag="ph0wf")
                nc.sync.dma_start(wc_f[:], din["wincT"][k])
                wc = ph0p.tile([128, H], F32R, tag="ph0w")
                nc.scalar.copy(wc[:], wc_f[:])
                nc.tensor.matmul(ps_h[:64, :], encT[:, k, :], wh[:],
                                 start=(k == 0), stop=False)
                nc.tensor.matmul(ps_c[:64, :], encT[:, k, :], wc[:],
                                 start=(k == 0), stop=False)
            nc.tensor.matmul(ps_h[:64, :], ones1[:, :64], bin_hr[:],
                             start=False, stop=True)
            nc.tensor.matmul(ps_c[:64, :], ones1[:, :64], bin_cr[:],
                             start=False, stop=True)
            c_sb = cstp.tile([64, H], F32, tag="c")
            nc.vector.tensor_copy(c_sb[:], ps_c[:64, :])
            h_sb = hstp.tile([64, H], F32, tag="h")
            nc.vector.tensor_copy(h_sb[:], ps_h[:64, :])

            h0T = constp.tile([128, KH, 64], F32R, tag="h0T")
            tp0 = ps2.tile([128, KH * 64], F32, tag="psA")
            for k in range(KH):
                nc.tensor.matmul(tp0[:, k * 64:(k + 1) * 64],
                                 h_sb[:, k * 128:(k + 1) * 128], ident[:],
                                 is_transpose=True,
                                 start=(k == 0), stop=(k == KH - 1))
            nc.vector.tensor_copy(h0T[:],
                                  tp0[:].rearrange("p (k b) -> p k b", k=KH))

        # ---- weights: DMA to fp32 staging, engine-copy to fp32r ---------
        w_hh_sb = constp.tile([128, KH, 4 * H], F32R, tag="whh")
        w_fc_sb = wfcp.tile([128, KH, VS], F32R, tag="wfc")
        w_ih_sb = ph1wp.tile([128, KH, 4 * H], F32R, tag="wih")
        for dst, src_, nch in ((w_hh_sb, din["w_hhT"], 8),
                               (w_ih_sb, din["w_ihT"], 8),
                               (w_fc_sb, din["w_fcT"], 15)):
            w = dst.shape[2] // nch
            for j in range(nch):
                stg = stagep.tile([128, KH, 256], F32, tag="stg")
                nc.sync.dma_start(stg[:, :, :w], src_[:, :, j * w:(j + 1) * w])
                nc.scalar.copy(dst[:, :, j * w:(j + 1) * w], stg[:, :, :w])

        stgb = stagep.tile([1, 4 * H], F32, tag="stgb", bufs=1)
        nc.sync.dma_start(stgb[:], din["bias_g"][:])
        bias_g = ph1wp.tile([1, 4 * H], F32R, tag="biasg")
        nc.vector.tensor_copy(bias_g[:], stgb[:])

        # ---- recurrence-era pools (reuse the ph0 zone) ------------------
        emtp = es.enter_context(pool(name="emt", bufs=2))
        xoutp = es.enter_context(pool(name="xout", bufs=2))
        xinp = es.enter_context(pool(name="xin", bufs=2))
        gatep = es.enter_context(pool(name="gate", bufs=1))
        hTp = es.enter_context(pool(name="hT", bufs=4))
        p2op = es.enter_context(pool(name="p2o", bufs=2))

        def phase1_tile(r):
            """X[128 rows, 2048] for step pair r -> DRAM scratch."""
            em_f = emtp.tile([128, KH, 128], F32, tag="emf", bufs=1)
            nc.sync.dma_start(em_f[:], din["embsT"][:, :, 128 * r:128 * (r + 1)])
            em = emtp.tile([128, KH, 128], F32R, tag="em")
            nc.vector.tensor_copy(em[:], em_f[:])
            for j in range(2):
                xo = xoutp.tile([128, 2, 512], F32, tag="xo")
                for h2 in range(2):
                    n = 2 * j + h2
                    ps = ps2.tile([128, 512], F32, tag="psA")
                    for k in range(KH):
                        nc.tensor.matmul(ps[:], em[:, k, :],
                                         w_ih_sb[:, k, 512 * n:512 * (n + 1)],
                                         start=(k == 0), stop=False)
                    nc.tensor.matmul(ps[:], ones1[:],
                                     bias_g[:, 512 * n:512 * (n + 1)],
                                     start=False, stop=True)
                    nc.vector.tensor_copy(xo[:, h2, :], ps[:])
                nc.sync.dma_start(
                    xdram[r][2 * j:2 * j + 2].rearrange("n p c -> p n c"),
                    xo[:])

        hT_tiles = {}
        state = {"h": h_sb, "c": c_sb}

        def step(s):
            """One LSTM step (batch-major); h -> feature-major pair tile."""
            if s % 2 == 0:
                hT_tiles[s // 2] = hTp.tile([128, KH, 128], F32R,
                                            tag="hTpair",
                                            name=f"hTpair{s // 2}")
            pair = hT_tiles[s // 2]
            half = (s % 2) * 64
            stat_t = h0T if s == 0 else hT_tiles[(s - 1) // 2]
            soff = 0 if s == 0 else ((s - 1) % 2) * 64
            stat = stat_t[:, :, soff:soff + 64] if stat_t is not h0T else h0T[:]

            xt = xinp.tile([64, KH, 512], F32, tag="xin")
            h64 = (s % 2) * 64
            nc.sync.dma_start(
                xt[:], xdram[s // 2][:, h64:h64 + 64, :].rearrange(
                    "n b c -> b n c"))

            # gate order: f, i, g, o  (w_hh columns: i|f|g|o)
            gps = {}
            for g in (1, 0, 2, 3):
                ps = ps4.tile([64, 512], F32, tag="gps")
                for k in range(KH):
                    nc.tensor.matmul(
                        ps[:], stat_t[:, k, soff:soff + 64],
                        w_hh_sb[:, k, 512 * g:512 * (g + 1)],
                        start=(k == 0), stop=(k == KH - 1))
                # X (with biases folded in) added on DVE straight into psum
                nc.vector.tensor_add(ps[:], ps[:], xt[:, g, :])
                gps[g] = ps

            c_sb = state["c"]
            f_sb = gatep.tile([64, H], F32, tag="f")
            nc.scalar.activation(f_sb[:], gps[1][:], AF.Sigmoid)
            i_sb = gatep.tile([64, H], F32, tag="i")
            nc.scalar.activation(i_sb[:], gps[0][:], AF.Sigmoid)
            g_sb = gatep.tile([64, H], F32, tag="g")
            nc.scalar.activation(g_sb[:], gps[2][:], AF.Tanh)
            o_sb = gatep.tile([64, H], F32, tag="o")
            nc.scalar.activation(o_sb[:], gps[3][:], AF.Sigmoid)

            nc.vector.tensor_mul(f_sb[:], f_sb[:], c_sb[:])   # f*c
            nc.vector.tensor_mul(i_sb[:], i_sb[:], g_sb[:])   # i*g
            c_new = cstp.tile([64, H], F32, tag="c")
            nc.vector.tensor_add(c_new[:], f_sb[:], i_sb[:])
            state["c"] = c_new
            nc.scalar.activation(g_sb[:], c_new[:], AF.Tanh)  # tanh(c)
            h_new = hstp.tile([64, H], F32, tag="h")
            nc.vector.tensor_mul(h_new[:], o_sb[:], g_sb[:])
            state["h"] = h_new

            tp = ps2.tile([128, KH * 64], F32, tag="psA")
            for k in range(KH):
                nc.tensor.matmul(tp[:, k * 64:(k + 1) * 64],
                                 h_new[:, k * 128:(k + 1) * 128], ident[:],
                                 is_transpose=True,
                                 start=(k == 0), stop=(k == KH - 1))
            src = tp[:].rearrange("p (k b) -> p k b", k=KH)
            nv = min(bv[s], 64)
            if nv >= 64:
                nc.vector.tensor_copy(pair[:, :, half:half + 64], src)
            else:
                if nv > 0:
                    nc.vector.tensor_copy(pair[:, :, half:half + nv],
                                          src[:, :, :nv])
                nc.gpsimd.memset(
                    pair[:, :, half + nv:half + 64].bitcast(F32), 0.0)

        def phase2_tile(r):
            """preds for steps 2r, 2r+1 against the vocab shard."""
            pair = hT_tiles.pop(r)
            for n in range(NVC):
                w, off = VCHS[n], VOFF[n]
                ps = psv.tile([128, 470], F32, tag="pv")
                for k in range(KH):
                    nc.tensor.matmul(
                        ps[:, :w], pair[:, k, :],
                        w_fc_sb[:, k, off:off + w],
                        start=(k == 0), stop=(k == KH - 1))
                ob = p2op.tile([128, 470], F32, tag="p2o")
                if n % 2 == 0:
                    nc.vector.tensor_copy(ob[:, :w], ps[:, :w])
                else:
                    nc.scalar.copy(ob[:, :w], ps[:, :w])
                for half in range(2):
                    s = 2 * r + half
                    nv = min(bv[s], 64)
                    if nv > 0:
                        nc.sync.dma_start(
                            out[s, 0:nv, off:off + w],
                            ob[64 * half:64 * half + nv, :w])

        # ---- emission: pipeline phase 1 / recurrence / phase 2 ----------
        phase1_tile(0)
        phase1_tile(1)
        for s in range(T):
            if s + 2 < R:
                phase1_tile(s + 2)
            step(s)
            if s >= 2 and s % 2 == 0:
                phase2_tile((s - 2) // 2)
        phase2_tile(R - 1)

    nc.compile()
    return nc


_CACHE = {}
LAST_RESULTS = None


def kernel(**inputs):
    x = {k: np.asarray(v) for k, v in inputs.items()}
    enc = np.ascontiguousarray(x["encoder_out"], dtype=np.float32)
    caps = x["encoded_captions"]
    lengths = x["caption_lengths"][:, 0]
    emb_w = np.ascontiguousarray(x["embedding_weight"], dtype=np.float32)
    w_ih = x["w_ih"].astype(np.float32, copy=False)
    b_ih = x["b_ih"].astype(np.float32, copy=False)
    w_hh = x["w_hh"].astype(np.float32, copy=False)
    b_hh = x["b_hh"].astype(np.float32, copy=False)
    w_init_h = x["w_init_h"].astype(np.float32, copy=False)
    b_init_h = x["b_init_h"].astype(np.float32, copy=False)
    w_init_c = x["w_init_c"].astype(np.float32, copy=False)
    b_init_c = x["b_init_c"].astype(np.float32, copy=False)
    w_fc = x["w_fc"].astype(np.float32, copy=False)
    b_fc = x["b_fc"].astype(np.float32, copy=False)

    sort_ind = np.argsort(-lengths.astype(np.int64), kind="stable")
    enc_s = enc[sort_ind]
    caps_s = caps[sort_ind]
    dec_len = (lengths[sort_ind].astype(np.int64) - 1)
    bv = [int((dec_len > s).sum()) for s in range(T)]

    toks = np.asarray(caps_s[:, :T], dtype=np.int64)
    embs = emb_w[toks]                                   # [B, T, E]
    em = np.ascontiguousarray(
        embs.transpose(1, 0, 2).reshape(ROWS, E))        # row t*64+b

    def kchunk(mat_t):  # [D, N] -> [128, D//128, N]
        d = mat_t.shape[0]
        return np.ascontiguousarray(
            mat_t.reshape(d // 128, 128, -1).transpose(1, 0, 2))

    feed = {
        "embsT": kchunk(em.T),
        "w_ihT": kchunk(w_ih.T),
        "w_hhT": kchunk(w_hh.T),
        "encT": kchunk(enc_s.T),
        "winhT": np.ascontiguousarray(w_init_h.T.reshape(KE, 128, H)),
        "wincT": np.ascontiguousarray(w_init_c.T.reshape(KE, 128, H)),
        "bias_g": (b_ih + b_hh).reshape(1, -1),
        "binh": b_init_h.reshape(1, -1),
        "binc": b_init_c.reshape(1, -1),
        "ident64": np.eye(64, dtype=np.float32),
    }
    feed = {k: np.ascontiguousarray(v, dtype=np.float32) for k, v in feed.items()}

    key = tuple(bv)
    if key not in _CACHE:
        _CACHE[key] = _build_program(bv)
    nc = _CACHE[key]

    in_maps = []
    for c in range(NCORES):
        m = dict(feed)
        m["w_fcT"] = kchunk(np.ascontiguousarray(w_fc[c * VS:(c + 1) * VS].T))
        in_maps.append(m)

    res = run_bass_kernel_spmd(nc, in_maps, core_ids=list(range(NCORES)))
    global LAST_RESULTS
    LAST_RESULTS = res
    shards = [res.results[c]["preds"].transpose(1, 0, 2)
              for c in range(NCORES)]
    preds = np.concatenate(shards, axis=2)

    if b_fc.any():
        mask = np.arange(T)[None, :] < dec_len[:, None]
        preds = preds + np.where(mask[:, :, None], b_fc[None, None, :], 0.0)

    int_dt = caps.dtype if caps.dtype in (np.int32, np.int64) else np.int64
    return (preds,
            caps_s.astype(int_dt, copy=False),
            dec_len.astype(x["caption_lengths"].dtype, copy=False),
            sort_ind.astype(np.int32))
